# revision 27
# baseline (speedup 1.0000x reference)
"""Trainium2 Bass kernel for nn_ExtractorMLP (GNN edge cosine-similarity).

Math:  out[e] = cos_sim(mlp(emb[col[e]]), mlp(emb[row[e]]))
where  mlp(x) = elu(x @ W1.T + b1) @ W2.T + b2   (b1 = b2 = 0 for this problem)

Strategy (edge-data-parallel, SWDGE-token-halving, 2D-sectioned pipeline):
  * Phase 1 (per node, replicated on every core): compute the normalized MLP
    output table  t[v] = g[v] / max(||g[v]||, eps)  in SBUF, node-major
    (node v in partition v%128, features at free cols [(v//128)*128, +128)),
    normalised per 512-node supertile so the table becomes usable as a
    growing prefix.
  * Phase 2 (edges, sharded 8 ways): edges are partitioned into sections
    (row-quarter rq x col-section cq); a section only touches table nodes
    below  need = max(row_end[rq], col_end[cq]).  Sections are emitted in
    ascending `need` order, interleaved with phase-1 supertiles, so phase 2
    streams concurrently with phase 1.
      - row side: dma_gather (SWDGE) of t[row] from the quarter's table
        slice with int16 quarter-local indices  -> f2 [128 feats, cols]
      - col side: NO gather.  Edges are grouped by col-block with group
        sizes equalized across cores (host-balanced, sizes baked into the
        program -> still SPMD); the host ships one-hot matrices and a
        per-block matmul  psF1 = table_block^T @ onehot  expands t[col] on
        the tensor engine (~0.7ns/token vs ~9ns/token on SWDGE).
      - prod = psF1 * f2 (DVE, PSUM x SBUF), then a sliding-one-hot reduce
        matmul contracts features into per-edge dots.

ELU identity used on device:  elu(x) = max(exp(min(x, 0)) - 1, x)
"""

import math

import numpy as np
import ml_dtypes

BF16 = ml_dtypes.bfloat16

H = 128          # feature dim
P = 128          # partitions
CHUNK = 512      # edges per reduce-matmul / PSUM bank width
GT = 4096        # max edges per dma_gather instruction
NCORES = 8
NSWQ = 1
ST_W = 512       # phase-1 supertile width (nodes)

_PROG_CACHE: dict = {}
LAST_RESULTS = None  # test harness can inspect exec_time_ns


def _row_bounds(n_pad):
    """Row segments, each <= 32767 (int16-indexable) and 128-aligned."""
    qb = [0, 2048, 6144, 12288, 24576, 36864]
    while n_pad - qb[-1] > 32640:
        qb.append(qb[-1] + 32640)
    qb.append(n_pad)
    return [b for b in qb if b <= n_pad]


def _col_bounds(n_pad):
    """Col sections (block-aligned); single section (col side ungated —
    expands are emitted after phase 1 anyway)."""
    return [0, n_pad]


def _build_program(n_pad, layout):
    import concourse.bacc as bacc
    import concourse.mybir as mybir
    import concourse.tile as tile
    from contextlib import ExitStack

    f32 = mybir.dt.float32
    bf16 = mybir.dt.bfloat16
    i16 = mybir.dt.int16
    Alu = mybir.AluOpType
    Act = mybir.ActivationFunctionType
    Axis = mybir.AxisListType

    # layout: ordered sections, each: (rq, need, start, end, groups)
    #   groups: tuple of (b, gstart, gsize)
    sections = layout
    rb = _row_bounds(n_pad)
    s_pad = sections[-1][3]
    n_chunks = s_pad // CHUNK
    n_groups_out = math.ceil(n_chunks / P)
    n_blocks = n_pad // H

    nc = bacc.Bacc("TRN2", target_bir_lowering=False, debug=False,
                   num_devices=NCORES, num_swdge_queues=NSWQ)

    embT = nc.dram_tensor("embT", [P, n_pad], bf16, kind="ExternalInput")
    w1t_d = nc.dram_tensor("w1t", [H, H], bf16, kind="ExternalInput")
    w2t_d = nc.dram_tensor("w2t", [H, H], bf16, kind="ExternalInput")
    oh_d = nc.dram_tensor("oh", [P, s_pad], bf16, kind="ExternalInput")
    ridx_d = nc.dram_tensor("ridx", [P, s_pad // 16], i16, kind="ExternalInput")
    out_d = nc.dram_tensor("out", [n_groups_out, P, CHUNK], f32,
                           kind="ExternalOutput")

    with ExitStack() as ctx:
        tc = ctx.enter_context(tile.TileContext(nc))
        const = ctx.enter_context(tc.tile_pool(name="const", bufs=1))
        p1 = ctx.enter_context(tc.tile_pool(name="p1", bufs=3))
        p2 = ctx.enter_context(tc.tile_pool(name="p2", bufs=5))
        post = ctx.enter_context(tc.tile_pool(name="post", bufs=2))
        poh = ctx.enter_context(tc.tile_pool(name="poh", bufs=2))
        pprod = ctx.enter_context(tc.tile_pool(name="pprod", bufs=GT // CHUNK))
        ps1 = ctx.enter_context(tc.tile_pool(name="ps1", bufs=2, space="PSUM"))
        ps2 = ctx.enter_context(tc.tile_pool(name="ps2", bufs=2, space="PSUM"))
        psf = ctx.enter_context(tc.tile_pool(name="psf", bufs=2, space="PSUM"))
        pso = ctx.enter_context(tc.tile_pool(name="pso", bufs=2, space="PSUM"))

        table = const.tile([P, n_pad], bf16, tag="table")
        w1t = const.tile([H, H], bf16, tag="w1t")
        w2t = const.tile([H, H], bf16, tag="w2t")
        onehot = const.tile([P, 2 * P - 1], bf16, tag="onehot")
        ss_all = const.tile([P, n_blocks], f32, tag="ss_all")
        r_all = const.tile([P, n_blocks], f32, tag="r_all")
        s_all = const.tile([P, n_blocks], f32, tag="s_all")
        m_all = const.tile([P, n_blocks], f32, tag="m_all")
        ridx = const.tile([P, s_pad // 16], i16, tag="ridx")
        nc.sync.dma_start(out=w1t[:], in_=w1t_d[:])
        nc.sync.dma_start(out=w2t[:], in_=w2t_d[:])
        nc.sync.dma_start(out=ridx[:], in_=ridx_d[:])
        nc.vector.memset(onehot[:], 0.0)
        nc.vector.memset(onehot[:, P - 1:P], 1.0)

        # ---------- phase-1 supertile (with in-supertile normalize) ----------
        def emit_supertile(n0, w):
            nb = w // H
            blk0 = n0 // H
            xt = p1.tile([P, ST_W], bf16, tag="xt", name="xt")[:, :w]
            nc.sync.dma_start(out=xt, in_=embT[:, n0:n0 + w])
            ph1 = ps1.tile([P, ST_W], f32, tag="ph1", name="ph1")[:, :w]
            nc.tensor.matmul(ph1, lhsT=w1t[:], rhs=xt, start=True, stop=True)
            u_t = p1.tile([P, ST_W], bf16, tag="u", name="u")[:, :w]
            nc.scalar.activation(u_t, ph1, Act.Relu, scale=-1.0)
            e_t = p1.tile([P, ST_W], bf16, tag="e", name="e")[:, :w]
            nc.scalar.activation(e_t, u_t, Act.Exp, scale=-1.0)
            h1_t = p1.tile([P, ST_W], bf16, tag="h1", name="h1")[:, :w]
            nc.vector.scalar_tensor_tensor(
                h1_t, in0=e_t, scalar=-1.0, in1=ph1,
                op0=Alu.add, op1=Alu.max)
            pg = ps2.tile([P, ST_W], f32, tag="pg", name="pg")[:, :w]
            for b in range(nb):
                nc.tensor.matmul(pg[:, b * H:(b + 1) * H],
                                 lhsT=h1_t[:, b * H:(b + 1) * H],
                                 rhs=w2t[:], start=True, stop=True)
            nc.scalar.activation(table[:, n0:n0 + w], pg, Act.Copy)
            # sum of squares on DVE (bf16 x bf16), windowed reduce
            sq_t = p1.tile([P, ST_W], bf16, tag="sq", name="sq")[:, :w]
            nc.vector.tensor_tensor(out=sq_t, in0=table[:, n0:n0 + w],
                                    in1=table[:, n0:n0 + w], op=Alu.mult)
            nc.vector.tensor_reduce(
                ss_all[:, blk0:blk0 + nb],
                sq_t.rearrange("p (b f) -> p b f", f=H),
                axis=Axis.X, op=Alu.add)

        # Sqrt lives in a different act-table set than Exp; batching the
        # normalization at section boundaries keeps table reloads rare.
        def emit_normalize(blo, bhi):
            if bhi <= blo:
                return
            nc.scalar.activation(s_all[:, blo:bhi], ss_all[:, blo:bhi],
                                 Act.Sqrt)
            nc.vector.tensor_scalar_max(m_all[:, blo:bhi],
                                        s_all[:, blo:bhi], 1e-8)
            nc.vector.reciprocal(r_all[:, blo:bhi], m_all[:, blo:bhi])
            for b in range(blo, bhi):
                nc.vector.tensor_scalar_mul(
                    table[:, b * H:(b + 1) * H],
                    table[:, b * H:(b + 1) * H],
                    r_all[:, b:b + 1])

        # ---------- phase-2: gathers (interleaved with phase 1) ----------
        def emit_section_gathers(sec):
            rq, need, start, end, groups = sec
            tslice = table[:, rb[rq]:rb[rq + 1]]
            tiles = []
            t0 = start
            while t0 < end:
                tsz = min(GT, end - t0)
                f2t = p2.tile([P, GT], bf16, tag="f2", name="f2")
                f2g = f2t[:, :tsz].rearrange("p (a t) -> p a t", a=1)
                nc.gpsimd.dma_gather(
                    f2g, tslice, ridx[:, t0 // 16:(t0 + tsz) // 16], tsz, tsz,
                    H, transpose=True, sbuf_tokens_per_rank=P,
                    sbuf_free_dim_per_rank=256, single_packet=False,
                    queue_num=0)
                tiles.append((t0, tsz, f2t))
                t0 += tsz
            return tiles

        # ---------- phase-2: expand / multiply / reduce (after phase 1) ----
        state = {"chunk_id": 0, "pout": None}

        def emit_section_math(sec, tiles):
            rq, need, start, end, groups = sec
            segs = [[] for _ in range((end - start) // CHUNK)]
            for (b, gstart, gsize) in groups:
                lo = gstart
                while lo < gstart + gsize:
                    c = (lo - start) // CHUNK
                    hi = min(gstart + gsize, start + (c + 1) * CHUNK)
                    segs[c].append((b, lo - start - c * CHUNK,
                                    hi - start - c * CHUNK))
                    lo = hi
            for (t0, tsz, f2t) in tiles:
                oht = poh.tile([P, GT], bf16, tag="oh", name="oh")
                nc.sync.dma_start(out=oht[:, :tsz], in_=oh_d[:, t0:t0 + tsz])
                prods = []
                for c0 in range(0, tsz, CHUNK):
                    c = (t0 - start + c0) // CHUNK
                    psF1 = psf.tile([P, CHUNK], f32, tag="psF1", name="psF1")
                    for (b, lo, hi) in segs[c]:
                        nc.tensor.matmul(
                            psF1[:, lo:hi],
                            lhsT=table[:, b * H:(b + 1) * H],
                            rhs=oht[:, c0 + lo:c0 + hi],
                            start=True, stop=True)
                    prod = pprod.tile([P, CHUNK], bf16, tag="prod")
                    nc.vector.tensor_tensor(
                        out=prod[:], in0=psF1[:],
                        in1=f2t[:, c0:c0 + CHUNK], op=Alu.mult)
                    prods.append(prod)
                for prod in prods:
                    cid = state["chunk_id"]
                    g, p = divmod(cid, P)
                    if p == 0:
                        state["pout"] = pso.tile([P, CHUNK], f32, tag="pout",
                                                 name="pout")
                    pout = state["pout"]
                    last = cid == n_chunks - 1
                    nc.tensor.matmul(pout[:],
                                     lhsT=onehot[:, P - 1 - p:2 * P - 1 - p],
                                     rhs=prod[:], start=(p == 0),
                                     stop=(p == P - 1 or last))
                    state["chunk_id"] = cid + 1
                    if p == P - 1 or last:
                        rows = p + 1
                        ost = post.tile([P, CHUNK], f32, tag="ost",
                                        name="ost")[:rows]
                        nc.vector.tensor_copy(out=ost, in_=pout[:rows])
                        nc.sync.dma_start(out=out_d[g, :rows], in_=ost)

        # ---------- interleaved emission ----------
        # gathers are gated on their row quarter (normalized); the col-side
        # math needs the whole table and is emitted after phase 1.
        sec_iter = iter(sections)
        next_sec = next(sec_iter, None)
        sec_tiles = []
        norm_upto = 0          # blocks normalized so far
        n0 = 0
        while n0 < n_pad:
            w = min(ST_W, n_pad - n0)
            emit_supertile(n0, w)
            n0 += w
            while next_sec is not None and rb[next_sec[0] + 1] <= n0:
                emit_normalize(norm_upto, rb[next_sec[0] + 1] // H)
                norm_upto = max(norm_upto, rb[next_sec[0] + 1] // H)
                sec_tiles.append((next_sec, emit_section_gathers(next_sec)))
                next_sec = next(sec_iter, None)
        emit_normalize(norm_upto, n_blocks)
        while next_sec is not None:
            sec_tiles.append((next_sec, emit_section_gathers(next_sec)))
            next_sec = next(sec_iter, None)
        for sec, tiles in sec_tiles:
            emit_section_math(sec, tiles)

    nc.compile()
    return nc


def _wrap_idx(idx):
    """[S*16] int16 -> [128, S] wrapped layout (16 partitions, replicated 8x)."""
    w = idx.reshape(-1, 16).T.astype(np.int16)
    return np.tile(w, (8, 1))


def _ensure_ntff_hook():
    """Provide antenv.axon_hooks if the image lacks it (trace support only)."""
    import sys
    import types
    try:
        import antenv.axon_hooks  # noqa: F401
        return
    except ImportError:
        pass
    try:
        import antenv
        from trn_agent_boot.trn_boot import _ntff_profile_via_ctypes
        mod = types.ModuleType("antenv.axon_hooks")
        mod._hook = _ntff_profile_via_ctypes("/opt/axon/libaxon_pjrt.so")
        mod.get_axon_ntff_profile_hook = lambda: mod._hook
        mod.set_axon_ntff_profile_hook = lambda h: setattr(mod, "_hook", h)
        sys.modules["antenv.axon_hooks"] = mod
        antenv.axon_hooks = mod
    except Exception:
        pass


def kernel(emb, edge_index, W1, b1, W2, b2):
    global LAST_RESULTS
    from concourse.bass_utils import run_bass_kernel_spmd
    _ensure_ntff_hook()

    emb = np.asarray(emb, dtype=np.float32)
    W1 = np.asarray(W1, dtype=np.float32)
    W2 = np.asarray(W2, dtype=np.float32)
    b1 = np.asarray(b1, dtype=np.float32)
    b2 = np.asarray(b2, dtype=np.float32)
    assert np.abs(b1).max() == 0 and np.abs(b2).max() == 0, \
        "nonzero biases not implemented"
    col = np.asarray(edge_index[0]).astype(np.int64)
    row = np.asarray(edge_index[1]).astype(np.int64)

    n, h = emb.shape
    assert h == H
    E = col.shape[0]
    n_pad = ((n + P - 1) // P) * P
    rb = np.asarray(_row_bounds(n_pad), dtype=np.int64)
    cb = np.asarray(_col_bounds(n_pad), dtype=np.int64)
    nrq = len(rb) - 1
    ncq = len(cb) - 1
    n_blocks = n_pad // H

    # ---- host prep: (section, col-block) groups, core-balanced ----
    rq_of = np.searchsorted(rb[1:-1], row, side="right")
    cq_of = np.searchsorted(cb[1:-1], col, side="right")
    b_of = col // H
    # section order: ascending need = max(row_end, col_end)
    sec_list = []
    for rq in range(nrq):
        for cq in range(ncq):
            need = max(int(rb[rq + 1]), int(cb[cq + 1]))
            sec_list.append((need, rq, cq))
    sec_list.sort()
    sec_rank = np.zeros((nrq, ncq), dtype=np.int64)
    for i, (need, rq, cq) in enumerate(sec_list):
        sec_rank[rq, cq] = i

    gkey = sec_rank[rq_of, cq_of] * n_blocks + b_of
    order = np.argsort(gkey, kind="stable")
    cnt = np.bincount(gkey, minlength=nrq * ncq * n_blocks)
    s_g = -(-cnt // NCORES)

    sections = []           # (rq, need, start, end, groups)
    gbase = np.full(nrq * ncq * n_blocks, -1, dtype=np.int64)
    pos = 0
    for i, (need, rq, cq) in enumerate(sec_list):
        start = pos
        groups = []
        for b in range(cb[cq] // H, cb[cq + 1] // H):
            g = i * n_blocks + b
            if s_g[g] == 0:
                continue
            groups.append([b, pos, int(s_g[g])])
            gbase[g] = pos
            pos += int(s_g[g])
        tail = (-pos) % CHUNK
        if tail:
            if groups:
                groups[-1][2] += tail
            else:
                groups.append([cb[cq] // H, pos, tail])
            pos += tail
        if pos > start:
            sections.append((rq, need, start, pos,
                             tuple((b, s, z) for (b, s, z) in groups)))
    s_pad = pos
    layout = tuple(sections)

    key = (n_pad, layout)
    if key not in _PROG_CACHE:
        _PROG_CACHE.clear()
        _PROG_CACHE[key] = _build_program(n_pad, layout)
    nc = _PROG_CACHE[key]

    # ---- per-core streams ----
    sorted_g = gkey[order]
    grp_changes = np.flatnonzero(np.diff(sorted_g, prepend=-1))
    grp_start_in_order = np.zeros_like(sorted_g)
    grp_start_in_order[grp_changes] = np.arange(len(order))[grp_changes]
    np.maximum.accumulate(grp_start_in_order, out=grp_start_in_order)
    rank = np.arange(len(order)) - grp_start_in_order
    core_of = rank % NCORES
    slot = rank // NCORES
    stream_pos = gbase[sorted_g] + slot

    embT = np.zeros((P, n_pad), dtype=BF16)
    embT[:, :n] = emb.T.astype(BF16)
    w1t = W1.T.astype(BF16)
    w2t = W2.T.astype(BF16)

    in_maps = []
    core_edge_ids = []
    core_positions = []
    for ci in range(NCORES):
        sel = core_of == ci
        eids = order[sel]
        pospc = stream_pos[sel]
        core_edge_ids.append(eids)
        core_positions.append(pospc)
        ohm = np.zeros((P, s_pad), dtype=BF16)
        ohm[col[eids] % H, pospc] = BF16(1.0)
        ridx_flat = np.zeros(s_pad, dtype=np.int64)
        ridx_flat[pospc] = row[eids] - rb[rq_of[eids]]
        in_maps.append({
            "embT": embT, "w1t": w1t, "w2t": w2t,
            "oh": ohm, "ridx": _wrap_idx(ridx_flat),
        })

    res = run_bass_kernel_spmd(nc, in_maps, core_ids=list(range(NCORES)))
    LAST_RESULTS = res

    # ---- reassemble ----
    out = np.empty(E, dtype=np.float32)
    for ci in range(NCORES):
        stream = res.results[ci]["out"].reshape(-1)
        out[core_edge_ids[ci]] = stream[core_positions[ci]]
    return out


# revision 28
# speedup vs baseline: 1.0734x; 1.0734x over previous
"""Trainium2 Bass kernel for nn_ExtractorMLP (GNN edge cosine-similarity).

Math:  out[e] = cos_sim(mlp(emb[col[e]]), mlp(emb[row[e]]))
where  mlp(x) = elu(x @ W1.T + b1) @ W2.T + b2   (b1 = b2 = 0 for this problem)

Strategy (edge-data-parallel, SWDGE-token-halving, 2D-sectioned pipeline):
  * Phase 1 (per node, replicated on every core): compute the normalized MLP
    output table  t[v] = g[v] / max(||g[v]||, eps)  in SBUF, node-major
    (node v in partition v%128, features at free cols [(v//128)*128, +128)),
    normalised per 512-node supertile so the table becomes usable as a
    growing prefix.
  * Phase 2 (edges, sharded 8 ways): edges are partitioned into sections
    (row-quarter rq x col-section cq); a section only touches table nodes
    below  need = max(row_end[rq], col_end[cq]).  Sections are emitted in
    ascending `need` order, interleaved with phase-1 supertiles, so phase 2
    streams concurrently with phase 1.
      - row side: dma_gather (SWDGE) of t[row] from the quarter's table
        slice with int16 quarter-local indices  -> f2 [128 feats, cols]
      - col side: NO gather.  Edges are grouped by col-block with group
        sizes equalized across cores (host-balanced, sizes baked into the
        program -> still SPMD); the host ships one-hot matrices and a
        per-block matmul  psF1 = table_block^T @ onehot  expands t[col] on
        the tensor engine (~0.7ns/token vs ~9ns/token on SWDGE).
      - prod = psF1 * f2 (DVE, PSUM x SBUF), then a sliding-one-hot reduce
        matmul contracts features into per-edge dots.

ELU identity used on device:  elu(x) = max(exp(min(x, 0)) - 1, x)
"""

import math

import numpy as np
import ml_dtypes

BF16 = ml_dtypes.bfloat16

H = 128          # feature dim
P = 128          # partitions
CHUNK = 512      # edges per reduce-matmul / PSUM bank width
GT = 2048        # max edges per dma_gather instruction
NCORES = 8
NSWQ = 1
ST_W = 512       # phase-1 supertile width (nodes)

_PROG_CACHE: dict = {}
LAST_RESULTS = None  # test harness can inspect exec_time_ns


def _row_bounds(n_pad):
    """Row segments, each <= 32767 (int16-indexable) and 128-aligned."""
    qb = [0, 4096, 12288, 24576, 36864]
    while n_pad - qb[-1] > 32640:
        qb.append(qb[-1] + 32640)
    qb.append(n_pad)
    return [b for b in qb if b <= n_pad]


def _col_bounds(n_pad):
    """Col sections (block-aligned); single section (col side ungated —
    expands are emitted after phase 1 anyway)."""
    return [0, n_pad]


def _build_program(n_pad, layout):
    import concourse.bacc as bacc
    import concourse.mybir as mybir
    import concourse.tile as tile
    from contextlib import ExitStack

    f32 = mybir.dt.float32
    bf16 = mybir.dt.bfloat16
    i16 = mybir.dt.int16
    Alu = mybir.AluOpType
    Act = mybir.ActivationFunctionType
    Axis = mybir.AxisListType

    # layout: ordered sections, each: (rq, need, start, end, groups)
    #   groups: tuple of (b, gstart, gsize)
    sections = layout
    rb = _row_bounds(n_pad)
    s_pad = sections[-1][3]
    n_chunks = s_pad // CHUNK
    n_groups_out = math.ceil(n_chunks / P)
    n_blocks = n_pad // H

    nc = bacc.Bacc("TRN2", target_bir_lowering=False, debug=False,
                   num_devices=NCORES, num_swdge_queues=NSWQ)

    embT = nc.dram_tensor("embT", [P, n_pad], bf16, kind="ExternalInput")
    w1t_d = nc.dram_tensor("w1t", [H, H], bf16, kind="ExternalInput")
    w2t_d = nc.dram_tensor("w2t", [H, H], bf16, kind="ExternalInput")
    oh_d = nc.dram_tensor("oh", [P, s_pad], bf16, kind="ExternalInput")
    ridx_d = nc.dram_tensor("ridx", [P, s_pad // 16], i16, kind="ExternalInput")
    out_d = nc.dram_tensor("out", [n_groups_out, P, CHUNK], f32,
                           kind="ExternalOutput")

    with ExitStack() as ctx:
        tc = ctx.enter_context(tile.TileContext(nc))
        const = ctx.enter_context(tc.tile_pool(name="const", bufs=1))
        p1 = ctx.enter_context(tc.tile_pool(name="p1", bufs=3))
        p2 = ctx.enter_context(tc.tile_pool(name="p2", bufs=10))
        post = ctx.enter_context(tc.tile_pool(name="post", bufs=2))
        poh = ctx.enter_context(tc.tile_pool(name="poh", bufs=4))
        pprod = ctx.enter_context(tc.tile_pool(name="pprod", bufs=2 * GT // CHUNK))
        ps1 = ctx.enter_context(tc.tile_pool(name="ps1", bufs=2, space="PSUM"))
        ps2 = ctx.enter_context(tc.tile_pool(name="ps2", bufs=2, space="PSUM"))
        psf = ctx.enter_context(tc.tile_pool(name="psf", bufs=3, space="PSUM"))
        pso = ctx.enter_context(tc.tile_pool(name="pso", bufs=1, space="PSUM"))

        table = const.tile([P, n_pad], bf16, tag="table")
        w1t = const.tile([H, H], bf16, tag="w1t")
        w2t = const.tile([H, H], bf16, tag="w2t")
        onehot = const.tile([P, 2 * P - 1], bf16, tag="onehot")
        ss_all = const.tile([P, n_blocks], f32, tag="ss_all")
        r_all = const.tile([P, n_blocks], f32, tag="r_all")
        s_all = const.tile([P, n_blocks], f32, tag="s_all")
        m_all = const.tile([P, n_blocks], f32, tag="m_all")
        ridx = const.tile([P, s_pad // 16], i16, tag="ridx")
        nc.sync.dma_start(out=w1t[:], in_=w1t_d[:])
        nc.sync.dma_start(out=w2t[:], in_=w2t_d[:])
        nc.sync.dma_start(out=ridx[:], in_=ridx_d[:])
        nc.vector.memset(onehot[:], 0.0)
        nc.vector.memset(onehot[:, P - 1:P], 1.0)

        # ---------- phase-1 supertile (with in-supertile normalize) ----------
        def emit_supertile(n0, w):
            nb = w // H
            blk0 = n0 // H
            xt = p1.tile([P, ST_W], bf16, tag="xt", name="xt")[:, :w]
            nc.sync.dma_start(out=xt, in_=embT[:, n0:n0 + w])
            ph1 = ps1.tile([P, ST_W], f32, tag="ph1", name="ph1")[:, :w]
            nc.tensor.matmul(ph1, lhsT=w1t[:], rhs=xt, start=True, stop=True)
            u_t = p1.tile([P, ST_W], bf16, tag="u", name="u")[:, :w]
            nc.scalar.activation(u_t, ph1, Act.Relu, scale=-1.0)
            e_t = p1.tile([P, ST_W], bf16, tag="e", name="e")[:, :w]
            nc.scalar.activation(e_t, u_t, Act.Exp, scale=-1.0)
            h1_t = p1.tile([P, ST_W], bf16, tag="h1", name="h1")[:, :w]
            nc.vector.scalar_tensor_tensor(
                h1_t, in0=e_t, scalar=-1.0, in1=ph1,
                op0=Alu.add, op1=Alu.max)
            pg = ps2.tile([P, ST_W], f32, tag="pg", name="pg")[:, :w]
            for b in range(nb):
                nc.tensor.matmul(pg[:, b * H:(b + 1) * H],
                                 lhsT=h1_t[:, b * H:(b + 1) * H],
                                 rhs=w2t[:], start=True, stop=True)
            nc.scalar.activation(table[:, n0:n0 + w], pg, Act.Copy)
            # sum of squares on DVE (bf16 x bf16), windowed reduce
            sq_t = p1.tile([P, ST_W], bf16, tag="sq", name="sq")[:, :w]
            nc.vector.tensor_tensor(out=sq_t, in0=table[:, n0:n0 + w],
                                    in1=table[:, n0:n0 + w], op=Alu.mult)
            nc.vector.tensor_reduce(
                ss_all[:, blk0:blk0 + nb],
                sq_t.rearrange("p (b f) -> p b f", f=H),
                axis=Axis.X, op=Alu.add)

        # Sqrt lives in a different act-table set than Exp; batching the
        # normalization at section boundaries keeps table reloads rare.
        def emit_normalize(blo, bhi):
            if bhi <= blo:
                return
            nc.scalar.activation(s_all[:, blo:bhi], ss_all[:, blo:bhi],
                                 Act.Sqrt)
            nc.vector.tensor_scalar_max(m_all[:, blo:bhi],
                                        s_all[:, blo:bhi], 1e-8)
            nc.vector.reciprocal(r_all[:, blo:bhi], m_all[:, blo:bhi])
            for b in range(blo, bhi):
                nc.vector.tensor_scalar_mul(
                    table[:, b * H:(b + 1) * H],
                    table[:, b * H:(b + 1) * H],
                    r_all[:, b:b + 1])

        # ---------- phase-2: gathers (interleaved with phase 1) ----------
        def emit_section_gathers(sec):
            rq, need, start, end, groups = sec
            tslice = table[:, rb[rq]:rb[rq + 1]]
            tiles = []
            t0 = start
            while t0 < end:
                tsz = min(GT, end - t0)
                f2t = p2.tile([P, GT], bf16, tag="f2", name="f2")
                f2g = f2t[:, :tsz].rearrange("p (a t) -> p a t", a=1)
                nc.gpsimd.dma_gather(
                    f2g, tslice, ridx[:, t0 // 16:(t0 + tsz) // 16], tsz, tsz,
                    H, transpose=True, sbuf_tokens_per_rank=P,
                    sbuf_free_dim_per_rank=256, single_packet=False,
                    queue_num=0)
                tiles.append((t0, tsz, f2t))
                t0 += tsz
            return tiles

        # ---------- phase-2: expand / multiply / reduce (after phase 1) ----
        state = {"chunk_id": 0, "pout": None}

        def emit_section_math(sec, tiles):
            rq, need, start, end, groups = sec
            segs = [[] for _ in range((end - start) // CHUNK)]
            for (b, gstart, gsize) in groups:
                lo = gstart
                while lo < gstart + gsize:
                    c = (lo - start) // CHUNK
                    hi = min(gstart + gsize, start + (c + 1) * CHUNK)
                    segs[c].append((b, lo - start - c * CHUNK,
                                    hi - start - c * CHUNK))
                    lo = hi
            for (t0, tsz, f2t) in tiles:
                oht = poh.tile([P, GT], bf16, tag="oh", name="oh")
                nc.sync.dma_start(out=oht[:, :tsz], in_=oh_d[:, t0:t0 + tsz])
                prods = []
                for c0 in range(0, tsz, CHUNK):
                    c = (t0 - start + c0) // CHUNK
                    psF1 = psf.tile([P, CHUNK], f32, tag="psF1", name="psF1")
                    for (b, lo, hi) in segs[c]:
                        nc.tensor.matmul(
                            psF1[:, lo:hi],
                            lhsT=table[:, b * H:(b + 1) * H],
                            rhs=oht[:, c0 + lo:c0 + hi],
                            start=True, stop=True)
                    prod = pprod.tile([P, CHUNK], bf16, tag="prod")
                    nc.vector.tensor_tensor(
                        out=prod[:], in0=psF1[:],
                        in1=f2t[:, c0:c0 + CHUNK], op=Alu.mult)
                    prods.append(prod)
                for prod in prods:
                    cid = state["chunk_id"]
                    g, p = divmod(cid, P)
                    if p == 0:
                        state["pout"] = pso.tile([P, CHUNK], f32, tag="pout",
                                                 name="pout")
                    pout = state["pout"]
                    last = cid == n_chunks - 1
                    nc.tensor.matmul(pout[:],
                                     lhsT=onehot[:, P - 1 - p:2 * P - 1 - p],
                                     rhs=prod[:], start=(p == 0),
                                     stop=(p == P - 1 or last))
                    state["chunk_id"] = cid + 1
                    if p == P - 1 or last:
                        rows = p + 1
                        ost = post.tile([P, CHUNK], f32, tag="ost",
                                        name="ost")[:rows]
                        nc.vector.tensor_copy(out=ost, in_=pout[:rows])
                        nc.sync.dma_start(out=out_d[g, :rows], in_=ost)

        # ---------- interleaved emission ----------
        # gathers are gated on their row quarter (normalized); the col-side
        # math needs the whole table and is emitted after phase 1.
        sec_iter = iter(sections)
        next_sec = next(sec_iter, None)
        sec_tiles = []
        norm_upto = 0          # blocks normalized so far
        n0 = 0
        while n0 < n_pad:
            w = min(ST_W, n_pad - n0)
            emit_supertile(n0, w)
            n0 += w
            while next_sec is not None and rb[next_sec[0] + 1] <= n0:
                emit_normalize(norm_upto, rb[next_sec[0] + 1] // H)
                norm_upto = max(norm_upto, rb[next_sec[0] + 1] // H)
                sec_tiles.append((next_sec, emit_section_gathers(next_sec)))
                next_sec = next(sec_iter, None)
        emit_normalize(norm_upto, n_blocks)
        while next_sec is not None:
            sec_tiles.append((next_sec, emit_section_gathers(next_sec)))
            next_sec = next(sec_iter, None)
        for sec, tiles in sec_tiles:
            emit_section_math(sec, tiles)

    nc.compile()
    return nc


def _wrap_idx(idx):
    """[S*16] int16 -> [128, S] wrapped layout (16 partitions, replicated 8x)."""
    w = idx.reshape(-1, 16).T.astype(np.int16)
    return np.tile(w, (8, 1))


def _ensure_ntff_hook():
    """Provide antenv.axon_hooks if the image lacks it (trace support only)."""
    import sys
    import types
    try:
        import antenv.axon_hooks  # noqa: F401
        return
    except ImportError:
        pass
    try:
        import antenv
        from trn_agent_boot.trn_boot import _ntff_profile_via_ctypes
        mod = types.ModuleType("antenv.axon_hooks")
        mod._hook = _ntff_profile_via_ctypes("/opt/axon/libaxon_pjrt.so")
        mod.get_axon_ntff_profile_hook = lambda: mod._hook
        mod.set_axon_ntff_profile_hook = lambda h: setattr(mod, "_hook", h)
        sys.modules["antenv.axon_hooks"] = mod
        antenv.axon_hooks = mod
    except Exception:
        pass


def kernel(emb, edge_index, W1, b1, W2, b2):
    global LAST_RESULTS
    from concourse.bass_utils import run_bass_kernel_spmd
    _ensure_ntff_hook()

    emb = np.asarray(emb, dtype=np.float32)
    W1 = np.asarray(W1, dtype=np.float32)
    W2 = np.asarray(W2, dtype=np.float32)
    b1 = np.asarray(b1, dtype=np.float32)
    b2 = np.asarray(b2, dtype=np.float32)
    assert np.abs(b1).max() == 0 and np.abs(b2).max() == 0, \
        "nonzero biases not implemented"
    col = np.asarray(edge_index[0]).astype(np.int64)
    row = np.asarray(edge_index[1]).astype(np.int64)

    n, h = emb.shape
    assert h == H
    E = col.shape[0]
    n_pad = ((n + P - 1) // P) * P
    rb = np.asarray(_row_bounds(n_pad), dtype=np.int64)
    cb = np.asarray(_col_bounds(n_pad), dtype=np.int64)
    nrq = len(rb) - 1
    ncq = len(cb) - 1
    n_blocks = n_pad // H

    # ---- host prep: (section, col-block) groups, core-balanced ----
    rq_of = np.searchsorted(rb[1:-1], row, side="right")
    cq_of = np.searchsorted(cb[1:-1], col, side="right")
    b_of = col // H
    # section order: ascending need = max(row_end, col_end)
    sec_list = []
    for rq in range(nrq):
        for cq in range(ncq):
            need = max(int(rb[rq + 1]), int(cb[cq + 1]))
            sec_list.append((need, rq, cq))
    sec_list.sort()
    sec_rank = np.zeros((nrq, ncq), dtype=np.int64)
    for i, (need, rq, cq) in enumerate(sec_list):
        sec_rank[rq, cq] = i

    gkey = sec_rank[rq_of, cq_of] * n_blocks + b_of
    order = np.argsort(gkey, kind="stable")
    cnt = np.bincount(gkey, minlength=nrq * ncq * n_blocks)
    s_g = -(-cnt // NCORES)

    sections = []           # (rq, need, start, end, groups)
    gbase = np.full(nrq * ncq * n_blocks, -1, dtype=np.int64)
    pos = 0
    for i, (need, rq, cq) in enumerate(sec_list):
        start = pos
        groups = []
        for b in range(cb[cq] // H, cb[cq + 1] // H):
            g = i * n_blocks + b
            if s_g[g] == 0:
                continue
            groups.append([b, pos, int(s_g[g])])
            gbase[g] = pos
            pos += int(s_g[g])
        tail = (-pos) % CHUNK
        if tail:
            if groups:
                groups[-1][2] += tail
            else:
                groups.append([cb[cq] // H, pos, tail])
            pos += tail
        if pos > start:
            sections.append((rq, need, start, pos,
                             tuple((b, s, z) for (b, s, z) in groups)))
    s_pad = pos
    layout = tuple(sections)

    key = (n_pad, layout)
    if key not in _PROG_CACHE:
        _PROG_CACHE.clear()
        _PROG_CACHE[key] = _build_program(n_pad, layout)
    nc = _PROG_CACHE[key]

    # ---- per-core streams ----
    sorted_g = gkey[order]
    grp_changes = np.flatnonzero(np.diff(sorted_g, prepend=-1))
    grp_start_in_order = np.zeros_like(sorted_g)
    grp_start_in_order[grp_changes] = np.arange(len(order))[grp_changes]
    np.maximum.accumulate(grp_start_in_order, out=grp_start_in_order)
    rank = np.arange(len(order)) - grp_start_in_order
    core_of = rank % NCORES
    slot = rank // NCORES
    stream_pos = gbase[sorted_g] + slot

    embT = np.zeros((P, n_pad), dtype=BF16)
    embT[:, :n] = emb.T.astype(BF16)
    w1t = W1.T.astype(BF16)
    w2t = W2.T.astype(BF16)

    in_maps = []
    core_edge_ids = []
    core_positions = []
    for ci in range(NCORES):
        sel = core_of == ci
        eids = order[sel]
        pospc = stream_pos[sel]
        core_edge_ids.append(eids)
        core_positions.append(pospc)
        ohm = np.zeros((P, s_pad), dtype=BF16)
        ohm[col[eids] % H, pospc] = BF16(1.0)
        ridx_flat = np.zeros(s_pad, dtype=np.int64)
        ridx_flat[pospc] = row[eids] - rb[rq_of[eids]]
        in_maps.append({
            "embT": embT, "w1t": w1t, "w2t": w2t,
            "oh": ohm, "ridx": _wrap_idx(ridx_flat),
        })

    res = run_bass_kernel_spmd(nc, in_maps, core_ids=list(range(NCORES)))
    LAST_RESULTS = res

    # ---- reassemble ----
    out = np.empty(E, dtype=np.float32)
    for ci in range(NCORES):
        stream = res.results[ci]["out"].reshape(-1)
        out[core_edge_ids[ci]] = stream[core_positions[ci]]
    return out
